# revision 14
# baseline (speedup 1.0000x reference)
"""Trainium2 Bass kernel for nn_AttentionBlock (GroupNorm + 4-head attention + proj + residual).

Sharding: 8 cores = (batch b in 0..3) x (head-pair p in 0..1).  Each core gets
x[b] and the weight slices for its two heads, computes GroupNorm -> QKV ->
attention -> partial proj (over its 128 attention-output channels), and returns
a partial [256, 4096] projection output.  The host sums the two partials per
batch, adds proj bias + residual, and reshapes.

The device program is identical on all cores (SPMD); all per-core variation is
carried by the input tensors.

Layout notes (per core):
  h   = groupnorm(x[b])                SBUF [c=128 x2, n=4096]   (in-place on x)
  Q2  = Wq_pair^T . h                  SBUF [128 (2 heads x 64 d), 4096]
  K2  = Wk_pair^T . h                  SBUF [128, 4096]
  V1  = [h^T . Wv_pair | ones]         SBUF [128 (m-chunk), 32, 2, 65]
  scores^T[m, n] = sum_d K[d,m] Q[d,n] via matmul(lhsT=K2[64 rows], rhs=Q2),
     two heads packed concurrently in PE row-groups (0,0) and (64,0).
  pexp = exp(0.125 * scores^T)         (ScalarE, no max-subtraction)
  pv[d+1, n] += V1[m-chunk]^T . pexp   accumulated over 32 m-chunks in PSUM;
     row 64 (the ones column) accumulates the softmax denominator.
  A = pv[0:64] * (1/pv[64]) broadcast  -> attention output [c, n] per head
  y_partial = Wp[:, pair]^T . A        [256, 4096] -> DRAM
"""

import numpy as np

import concourse.bacc as bacc
import concourse.bass as bass
import concourse.mybir as mybir
import concourse.tile as tile

B = 4
C = 256
N = 4096          # 64*64
NH = 4
D = 64            # head dim
GROUPS = 32
EPS = 1e-5
NCORES = 8
SCALE = float(D) ** -0.5  # 0.125
F32 = mybir.dt.float32

MB = 32           # m chunks of 128
NB = 8            # n chunks of 512

F32R = mybir.dt.float32r


def _cast(ap, use_f32r):
    """Bitcast an fp32 AP to float32r (walrus's fast fp32 matmul mode)."""
    return ap.bitcast(F32R) if use_f32r else ap


def _build_program(has_v_bias: bool, f32r_attn: bool = False, f32r_mm: bool = False):
    nc = bacc.Bacc("TRN2", target_bir_lowering=False)
    MMD = F32R if f32r_mm else F32      # dtype of tiles feeding qkv/proj matmuls
    MAD = F32R if f32r_attn else F32    # dtype of tiles feeding QK/PV matmuls

    xb = nc.dram_tensor("xb", [C, N], F32, kind="ExternalInput")
    wqkT = nc.dram_tensor("wqkT", [C, 256], F32, kind="ExternalInput")   # cols 0:128 Qpair, 128:256 Kpair
    wvT = nc.dram_tensor("wvT", [C, 128], F32, kind="ExternalInput")
    wpT = nc.dram_tensor("wpT", [128, C], F32, kind="ExternalInput")     # rows = pair channels
    gnw = nc.dram_tensor("gnw", [C], F32, kind="ExternalInput")
    gnb = nc.dram_tensor("gnb", [C], F32, kind="ExternalInput")
    qkb = nc.dram_tensor("qkb", [256], F32, kind="ExternalInput")        # 0:128 Q bias pair, 128:256 K bias pair
    if has_v_bias:
        vb = nc.dram_tensor("vb", [128], F32, kind="ExternalInput")
    yp = nc.dram_tensor("yp", [C, N], F32, kind="ExternalOutput")

    # group-indicator matrix: G[c, g] = 1 if c // 8 == g else 0  (per c-tile)
    g_host = np.zeros((128, 16), dtype=np.float32)
    for c in range(128):
        g_host[c, c // 8] = 1.0
    g_dram = nc.inline_tensor(g_host, name="gmat")

    with tile.TileContext(nc) as tc:
        with (
            tc.tile_pool(name="consts", bufs=1) as consts,
            tc.tile_pool(name="xh", bufs=2) as xh_pool,
            tc.tile_pool(name="hp", bufs=2) as hp_pool,
            tc.tile_pool(name="qk_sb", bufs=1) as qk_sb,
            tc.tile_pool(name="v1p", bufs=1) as v1p,
            tc.tile_pool(name="st", bufs=2) as st,
            tc.tile_pool(name="pexpp", bufs=3) as pexpp,
            tc.tile_pool(name="recp", bufs=4) as recp,
            tc.tile_pool(name="drec", bufs=4, space="DRAM") as drec,
            tc.tile_pool(name="rbp", bufs=4) as rbp,
            tc.tile_pool(name="yout", bufs=4) as yout,
        ):
            # ---- constants ----
            g_sb = consts.tile([128, 16], F32, tag="gsb")
            nc.sync.dma_start(out=g_sb, in_=g_dram[:, :])
            def load_weight(shape, dram_slice, tag):
                w_raw = consts.tile(shape, F32, tag=tag + "_r", name=tag + "_r")
                nc.sync.dma_start(out=w_raw, in_=dram_slice)
                if not f32r_mm:
                    return w_raw
                w_c = consts.tile(shape, MMD, tag=tag, name=tag)
                nc.vector.tensor_copy(w_c, w_raw)
                return w_c

            w_qk = [load_weight([128, 256], wqkT[t * 128:(t + 1) * 128, :], f"wqk{t}")
                    for t in range(2)]
            w_v = [load_weight([128, 128], wvT[t * 128:(t + 1) * 128, :], f"wv{t}")
                   for t in range(2)]
            wp_lo = load_weight([64, 256], wpT[0:64, :], "wplo")
            wp_hi = load_weight([64, 256], wpT[64:128, :], "wphi")

            gw_t, gb_t = [], []
            for t in range(2):
                gwt = consts.tile([128, 1], F32, tag=f"gw{t}")
                nc.sync.dma_start(out=gwt, in_=gnw[t * 128:(t + 1) * 128])
                gw_t.append(gwt)
                gbt = consts.tile([128, 1], F32, tag=f"gb{t}")
                nc.sync.dma_start(out=gbt, in_=gnb[t * 128:(t + 1) * 128])
                gb_t.append(gbt)
            bq = consts.tile([128, 1], F32, tag="bq")
            nc.sync.dma_start(out=bq, in_=qkb[0:128])
            bk = consts.tile([128, 1], F32, tag="bk")
            nc.sync.dma_start(out=bk, in_=qkb[128:256])
            if has_v_bias:
                vb_sb = consts.tile([128, 128], F32, tag="vbsb")
                nc.sync.dma_start(
                    out=vb_sb,
                    in_=bass.AP(tensor=vb, offset=0, ap=[[0, 128], [1, 128]]),
                )
            eps16 = consts.tile([16, 1], F32, tag="eps16")
            nc.vector.memset(eps16, EPS)

            # ---- phase A: GroupNorm (in place: x tile becomes h tile) ----
            h_t = []
            scale_t, bias_t = [], []
            with tc.tile_pool(name="ps_g", bufs=2, space="PSUM") as ps_g:
                for t in range(2):
                    x_t = xh_pool.tile([128, N], F32, tag="x", name=f"x{t}")
                    nc.sync.dma_start(out=x_t, in_=xb[t * 128:(t + 1) * 128, :])
                    h_t.append(x_t)

                    stats = st.tile([128, 8, 6], F32, tag="stats")
                    xr = x_t.rearrange("p (k f) -> p k f", f=512)
                    for k in range(8):
                        nc.vector.bn_stats(out=stats[:, k, :], in_=xr[:, k, :])
                    mv = st.tile([128, 2], F32, tag="mv")
                    nc.vector.bn_aggr(out=mv, in_=stats)

                    # mq = [mean_c, var_c + mean_c^2]
                    mq = st.tile([128, 2], F32, tag="mq")
                    nc.vector.tensor_copy(mq[:, 0:1], mv[:, 0:1])
                    sq = st.tile([128, 1], F32, tag="sq")
                    nc.vector.tensor_mul(sq, mv[:, 0:1], mv[:, 0:1])
                    nc.vector.tensor_add(mq[:, 1:2], mv[:, 1:2], sq)

                    gps = ps_g.tile([16, 2], F32, tag="gps")
                    nc.tensor.matmul(gps, lhsT=g_sb, rhs=mq, start=True, stop=True)
                    # per-group E[x], E[x^2]
                    gsb = st.tile([16, 2], F32, tag="gsb2")
                    nc.scalar.mul(gsb, gps, 0.125)
                    sqg = st.tile([16, 1], F32, tag="sqg")
                    nc.vector.tensor_mul(sqg, gsb[:, 0:1], gsb[:, 0:1])
                    var = st.tile([16, 1], F32, tag="var")
                    nc.vector.tensor_sub(var, gsb[:, 1:2], sqg)
                    std = st.tile([16, 1], F32, tag="std")
                    nc.scalar.activation(std, var, mybir.ActivationFunctionType.Sqrt,
                                         bias=eps16, scale=1.0)
                    rstd = st.tile([16, 1], F32, tag="rstd")
                    nc.vector.reciprocal(rstd, std)
                    ms = st.tile([16, 2], F32, tag=f"ms{t}")
                    nc.vector.tensor_copy(ms[:, 0:1], gsb[:, 0:1])
                    nc.vector.tensor_copy(ms[:, 1:2], rstd)

                    # broadcast per-group stats to per-channel [128,1] via DMA
                    bmean = st.tile([128, 1], F32, tag=f"bmean{t}")
                    src = ms[:, 0:1]
                    nc.sync.dma_start(
                        out=bmean,
                        in_=bass.AP(tensor=src.tensor, offset=src.offset,
                                    ap=[src.ap[0], [0, 8]]),
                    )
                    brstd = st.tile([128, 1], F32, tag=f"brstd{t}")
                    src2 = ms[:, 1:2]
                    nc.sync.dma_start(
                        out=brstd,
                        in_=bass.AP(tensor=src2.tensor, offset=src2.offset,
                                    ap=[src2.ap[0], [0, 8]]),
                    )
                    sc = consts.tile([128, 1], F32, tag=f"scale{t}")
                    nc.vector.tensor_mul(sc, brstd, gw_t[t])
                    scale_t.append(sc)
                    tmp = st.tile([128, 1], F32, tag="tmpb")
                    nc.vector.tensor_mul(tmp, bmean, sc)
                    bi = consts.tile([128, 1], F32, tag=f"bias{t}")
                    nc.vector.tensor_sub(bi, gb_t[t], tmp)
                    bias_t.append(bi)

                x_t_list = h_t
                h_t = []
                for t in range(2):
                    # h = x * scale + bias
                    h_new = hp_pool.tile([128, N], MMD, tag="h", name=f"h{t}")
                    nc.vector.tensor_scalar(
                        out=h_new, in0=x_t_list[t],
                        scalar1=scale_t[t], scalar2=bias_t[t],
                        op0=mybir.AluOpType.mult, op1=mybir.AluOpType.add,
                    )
                    h_t.append(h_new)

            # ---- phase B: QKV ----
            Q2 = qk_sb.tile([128, N], MAD, tag="q2")
            K2 = qk_sb.tile([128, N], MAD, tag="k2")
            V1 = v1p.tile([128, MB, 2, 65], MAD, tag="v1")
            with tc.tile_pool(name="ps_qkv", bufs=3, space="PSUM") as ps_qkv:
                for dst, col0, bias_ap in ((Q2, 0, bq), (K2, 128, bk)):
                    for nb in range(NB):
                        ps = ps_qkv.tile([128, 512], F32, tag="mm")
                        ns = slice(nb * 512, (nb + 1) * 512)
                        nc.tensor.matmul(ps, lhsT=_cast(w_qk[0][:, col0:col0 + 128], f32r_mm),
                                         rhs=_cast(h_t[0][:, ns], f32r_mm), start=True, stop=False)
                        nc.tensor.matmul(ps, lhsT=_cast(w_qk[1][:, col0:col0 + 128], f32r_mm),
                                         rhs=_cast(h_t[1][:, ns], f32r_mm), start=False, stop=True)
                        nc.vector.tensor_scalar(
                            out=dst[:, ns], in0=ps, scalar1=bias_ap, scalar2=None,
                            op0=mybir.AluOpType.add,
                        )
                ones1 = consts.tile([128, 1], F32, tag="ones1")
                nc.vector.memset(ones1, 1.0)
                nc.vector.tensor_copy(
                    V1[:, :, :, 64:65].rearrange("p a b o -> p (a b o)"),
                    ones1.to_broadcast((128, MB * 2)),
                )
                for ch in range(MB):
                    ps = ps_qkv.tile([128, 128], F32, tag="mm")
                    cs = slice(ch * 128, (ch + 1) * 128)
                    nc.tensor.matmul(ps, lhsT=_cast(h_t[0][:, cs], f32r_mm),
                                     rhs=w_v[0], start=True, stop=False)
                    nc.tensor.matmul(ps, lhsT=_cast(h_t[1][:, cs], f32r_mm),
                                     rhs=w_v[1], start=False, stop=True)
                    if has_v_bias:
                        nc.vector.tensor_add(
                            V1[:, ch, :, 0:64],
                            ps.rearrange("p (h d) -> p h d", h=2),
                            vb_sb.rearrange("p (h d) -> p h d", h=2),
                        )
                    else:
                        nc.vector.tensor_copy(
                            V1[:, ch, :, 0:64],
                            ps.rearrange("p (h d) -> p h d", h=2),
                        )

            # ---- phase C: attention ----
            A_lo = xh_pool.tile([64, N], MMD, tag="x", name="A_lo")
            A_hi = xh_pool.tile([64, N], MMD, tag="x", name="A_hi")
            with (
                tc.tile_pool(name="ps_qk", bufs=2, space="PSUM") as ps_qk,
                tc.tile_pool(name="ps_pv", bufs=4, space="PSUM") as ps_pv,
            ):
                def emit_qk(nb, mb):
                    ns = slice(nb * 512, (nb + 1) * 512)
                    ms_ = slice(mb * 128, (mb + 1) * 128)
                    qk = ps_qk.tile([128, 1024], F32, tag="qk", name=f"qk_{nb}_{mb}")
                    nc.tensor.matmul(qk[:, 0:512], lhsT=_cast(K2[0:64, ms_], f32r_attn),
                                     rhs=_cast(Q2[0:64, ns], f32r_attn), start=True, stop=True,
                                     skip_group_check=True)
                    nc.tensor.matmul(qk[:, 512:1024], lhsT=_cast(K2[64:128, ms_], f32r_attn),
                                     rhs=_cast(Q2[64:128, ns], f32r_attn), start=True, stop=True,
                                     skip_group_check=True)
                    return qk

                # software pipeline: emit iteration i+1's QK matmuls before
                # iteration i's PV matmuls, so the in-order PE queue never
                # stalls behind a PV that waits on ScalarE's exp.
                iters = [(nb, mb) for nb in range(NB) for mb in range(MB)]
                pv_tiles = {}
                qk_cur = emit_qk(*iters[0])
                for idx, (nb, mb) in enumerate(iters):
                    ns = slice(nb * 512, (nb + 1) * 512)
                    if mb == 0:
                        pv_lo = ps_pv.tile([65, 512], F32, tag="pv", name=f"pvlo_{nb}")
                        pv_hi = ps_pv.tile([65, 512], F32, tag="pv", name=f"pvhi_{nb}")
                        pv_tiles[nb] = (pv_lo, pv_hi)
                    pv_lo, pv_hi = pv_tiles[nb]
                    pexp = pexpp.tile([128, 1024], MAD, tag="pexp", name=f"pexp_{nb}_{mb}")
                    nc.scalar.activation(pexp, qk_cur,
                                         mybir.ActivationFunctionType.Exp,
                                         scale=SCALE)
                    if idx + 1 < len(iters):
                        qk_cur = emit_qk(*iters[idx + 1])
                    nc.tensor.matmul(pv_lo, lhsT=_cast(V1[:, mb, 0, :], f32r_attn),
                                     rhs=_cast(pexp[:, 0:512], f32r_attn),
                                     start=(mb == 0), stop=(mb == MB - 1),
                                     skip_group_check=True)
                    nc.tensor.matmul(pv_hi, lhsT=_cast(V1[:, mb, 1, :], f32r_attn),
                                     rhs=_cast(pexp[:, 512:1024], f32r_attn),
                                     start=(mb == 0), stop=(mb == MB - 1),
                                     skip_group_check=True)
                    if mb == MB - 1:
                        # normalize: A = pv[0:64] / pv[64]
                        for pv, A in ((pv_lo, A_lo), (pv_hi, A_hi)):
                            rec = recp.tile([65, 512], F32, tag="rec")
                            nc.vector.reciprocal(rec[64:65, :], pv[64:65, :])
                            # bounce through DRAM: SBUF APs cannot have a
                            # zero-step partition dim, DRAM APs can.
                            dr = drec.tile([1, 512], F32, tag="dr")
                            nc.sync.dma_start(out=dr, in_=rec[64:65, :])
                            rb = rbp.tile([64, 512], F32, tag="rb")
                            nc.sync.dma_start(
                                out=rb,
                                in_=bass.AP(tensor=dr.tensor, offset=dr.offset,
                                            ap=[[0, 64]] + list(dr.ap[1:])),
                            )
                            nc.vector.tensor_mul(A[:, ns], pv[0:64, :], rb)

            # ---- phase D: partial proj ----
            with tc.tile_pool(name="ps_pj", bufs=2, space="PSUM") as ps_pj:
                for m in range(2):
                    for nb in range(NB):
                        ns = slice(nb * 512, (nb + 1) * 512)
                        ps = ps_pj.tile([128, 512], F32, tag="pj")
                        nc.tensor.matmul(ps, lhsT=_cast(wp_lo[:, m * 128:(m + 1) * 128], f32r_mm),
                                         rhs=_cast(A_lo[:, ns], f32r_mm), start=True, stop=False)
                        nc.tensor.matmul(ps, lhsT=_cast(wp_hi[:, m * 128:(m + 1) * 128], f32r_mm),
                                         rhs=_cast(A_hi[:, ns], f32r_mm), start=False, stop=True)
                        y_sb = yout.tile([128, 512], F32, tag="y")
                        nc.scalar.copy(y_sb, ps)
                        nc.sync.dma_start(out=yp[m * 128:(m + 1) * 128, ns], in_=y_sb)

    nc.finalize()
    return nc


_CACHE = {}


F32R_ATTN = True
F32R_MM = True


def _get_program(has_v_bias: bool):
    key = ("prog", has_v_bias, F32R_ATTN, F32R_MM)
    if key not in _CACHE:
        _CACHE[key] = _build_program(has_v_bias, F32R_ATTN, F32R_MM)
    return _CACHE[key]


def _make_in_maps(x, gn_w, gn_b, qkv_w, qkv_b, proj_w):
    x = np.ascontiguousarray(x, dtype=np.float32)
    in_maps = []
    for core in range(NCORES):
        b, p = core // 2, core % 2
        rows_q = slice(p * 128, (p + 1) * 128)
        rows_k = slice(256 + p * 128, 256 + (p + 1) * 128)
        rows_v = slice(512 + p * 128, 512 + (p + 1) * 128)
        m = {
            "xb": np.ascontiguousarray(x[b].reshape(C, N)),
            "wqkT": np.ascontiguousarray(
                np.concatenate([qkv_w[rows_q], qkv_w[rows_k]], axis=0).T.astype(np.float32)),
            "wvT": np.ascontiguousarray(qkv_w[rows_v].T.astype(np.float32)),
            "wpT": np.ascontiguousarray(proj_w[:, p * 128:(p + 1) * 128].T.astype(np.float32)),
            "gnw": np.ascontiguousarray(gn_w.astype(np.float32)),
            "gnb": np.ascontiguousarray(gn_b.astype(np.float32)),
            "qkb": np.ascontiguousarray(
                np.concatenate([qkv_b[rows_q], qkv_b[rows_k]]).astype(np.float32)),
        }
        if np.any(qkv_b[512:768]):
            m["vb"] = np.ascontiguousarray(qkv_b[rows_v].astype(np.float32))
        in_maps.append(m)
    return in_maps


def _get_executor(nc):
    """Build (once) a cached jitted 8-core executor for the program.

    Mirrors concourse.bass2jax.run_bass_via_pjrt, but caches the jitted
    callable so repeat kernel() calls don't re-trace/re-compile the XLA
    wrapper.  Returns (fn, in_names, out_names) where fn takes a list of
    per-core input dicts and returns a list of per-core output dicts.
    """
    key = ("exec", id(nc))
    if key in _CACHE:
        return _CACHE[key]
    import jax
    import concourse.mybir as _mybir
    from jax.experimental.shard_map import shard_map
    from jax.sharding import Mesh, PartitionSpec
    from concourse import bass2jax

    bass2jax.install_neuronx_cc_hook()
    partition_name = nc.partition_id_tensor.name if nc.partition_id_tensor else None
    in_names, out_names, out_avals, zero_outs = [], [], [], []
    for alloc in nc.m.functions[0].allocations:
        if not isinstance(alloc, _mybir.MemoryLocationSet):
            continue
        name = alloc.memorylocations[0].name
        if alloc.kind == "ExternalInput":
            if name != partition_name:
                in_names.append(name)
        elif alloc.kind == "ExternalOutput":
            shape = tuple(alloc.tensor_shape)
            dtype = _mybir.dt.np(alloc.dtype)
            out_names.append(name)
            out_avals.append(jax.core.ShapedArray(shape, dtype))
            zero_outs.append(np.zeros(shape, dtype))
    n_params = len(in_names)
    n_outs = len(out_avals)
    all_names = in_names + out_names + ([partition_name] if partition_name else [])

    def _body(*args):
        operands = list(args)
        if partition_name is not None:
            operands.append(bass2jax.partition_id_tensor())
        outs = bass2jax._bass_exec_p.bind(
            *operands,
            out_avals=tuple(out_avals),
            in_names=tuple(all_names),
            out_names=tuple(out_names),
            lowering_input_output_aliases=(),
            sim_require_finite=True,
            sim_require_nnan=True,
            nc=nc,
        )
        return tuple(outs)

    devices = jax.devices()[:NCORES]
    mesh = Mesh(np.asarray(devices), ("core",))
    in_specs = (PartitionSpec("core"),) * (n_params + n_outs)
    out_specs = (PartitionSpec("core"),) * n_outs
    donate = tuple(range(n_params, n_params + n_outs))
    sharded = jax.jit(
        shard_map(_body, mesh=mesh, in_specs=in_specs, out_specs=out_specs,
                  check_rep=False),
        donate_argnums=donate, keep_unused=True,
    )

    def fn(in_maps):
        concat_in = [
            np.concatenate([np.asarray(in_maps[c][nm]) for c in range(NCORES)], axis=0)
            for nm in in_names
        ]
        concat_zeros = [
            np.zeros((NCORES * z.shape[0], *z.shape[1:]), z.dtype) for z in zero_outs
        ]
        out_arrs = sharded(*concat_in, *concat_zeros)
        return [
            {nm: np.asarray(out_arrs[i]).reshape(NCORES, *out_avals[i].shape)[c]
             for i, nm in enumerate(out_names)}
            for c in range(NCORES)
        ]

    _CACHE[key] = (fn, in_names, out_names)
    return _CACHE[key]


def run(inputs, trace=False):
    """Run the sharded kernel.  Returns (output, BassKernelResults)."""
    from concourse.bass_utils import run_bass_kernel_spmd

    x = np.asarray(inputs["x"], dtype=np.float32)
    gn_w = np.asarray(inputs["gn_w"], dtype=np.float32)
    gn_b = np.asarray(inputs["gn_b"], dtype=np.float32)
    qkv_w = np.asarray(inputs["qkv_w"], dtype=np.float32)
    qkv_b = np.asarray(inputs["qkv_b"], dtype=np.float32)
    proj_w = np.asarray(inputs["proj_w"], dtype=np.float32)
    proj_b = np.asarray(inputs["proj_b"], dtype=np.float32)

    has_v_bias = bool(np.any(qkv_b[512:768]))
    nc = _get_program(has_v_bias)
    in_maps = _make_in_maps(x, gn_w, gn_b, qkv_w, qkv_b, proj_w)
    res = run_bass_kernel_spmd(nc, in_maps, list(range(NCORES)), trace=trace)
    parts = [res.results[c]["yp"] for c in range(NCORES)]
    y = np.stack([parts[2 * b] + parts[2 * b + 1] for b in range(B)])  # [B, C, N]
    y = y + proj_b[None, :, None]
    out = x + y.reshape(B, C, 64, 64)
    return out.astype(np.float32), res


def kernel(**inputs) -> np.ndarray:
    out, _ = run(inputs, trace=False)
    return out


# revision 18
# speedup vs baseline: 1.0733x; 1.0733x over previous
"""Trainium2 Bass kernel for nn_AttentionBlock (GroupNorm + 4-head attention + proj + residual).

Sharding: 8 cores = (batch b in 0..3) x (head-pair p in 0..1).  Each core gets
x[b] and the weight slices for its two heads, computes GroupNorm -> QKV ->
attention -> partial proj (over its 128 attention-output channels), and returns
a partial [256, 4096] projection output.  The host sums the two partials per
batch, adds proj bias + residual, and reshapes.

The device program is identical on all cores (SPMD); all per-core variation is
carried by the input tensors.

Layout notes (per core):
  h   = groupnorm(x[b])                SBUF [c=128 x2, n=4096]   (in-place on x)
  Q2  = Wq_pair^T . h                  SBUF [128 (2 heads x 64 d), 4096]
  K2  = Wk_pair^T . h                  SBUF [128, 4096]
  V1  = [h^T . Wv_pair | ones]         SBUF [128 (m-chunk), 32, 2, 65]
  scores^T[m, n] = sum_d K[d,m] Q[d,n] via matmul(lhsT=K2[64 rows], rhs=Q2),
     two heads packed concurrently in PE row-groups (0,0) and (64,0).
  pexp = exp(0.125 * scores^T)         (ScalarE, no max-subtraction)
  pv[d+1, n] += V1[m-chunk]^T . pexp   accumulated over 32 m-chunks in PSUM;
     row 64 (the ones column) accumulates the softmax denominator.
  A = pv[0:64] * (1/pv[64]) broadcast  -> attention output [c, n] per head
  y_partial = Wp[:, pair]^T . A        [256, 4096] -> DRAM
"""

import numpy as np

import concourse.bacc as bacc
import concourse.bass as bass
import concourse.mybir as mybir
import concourse.tile as tile

B = 4
C = 256
N = 4096          # 64*64
NH = 4
D = 64            # head dim
GROUPS = 32
EPS = 1e-5
NCORES = 8
SCALE = float(D) ** -0.5  # 0.125
F32 = mybir.dt.float32

MB = 32           # m chunks of 128
NB = 8            # n chunks of 512

F32R = mybir.dt.float32r


def _cast(ap, use_f32r):
    """Bitcast an fp32 AP to float32r (walrus's fast fp32 matmul mode)."""
    return ap.bitcast(F32R) if use_f32r else ap


def _build_program(has_v_bias: bool, f32r_attn: bool = False, f32r_mm: bool = False):
    nc = bacc.Bacc("TRN2", target_bir_lowering=False)
    MMD = F32R if f32r_mm else F32      # dtype of tiles feeding qkv/proj matmuls
    MAD = F32R if f32r_attn else F32    # dtype of tiles feeding QK/PV matmuls

    xb = nc.dram_tensor("xb", [C, N], F32, kind="ExternalInput")
    wqkT = nc.dram_tensor("wqkT", [C, 256], F32, kind="ExternalInput")   # cols 0:128 Qpair, 128:256 Kpair
    wvT = nc.dram_tensor("wvT", [C, 128], F32, kind="ExternalInput")
    wpT = nc.dram_tensor("wpT", [128, C], F32, kind="ExternalInput")     # rows = pair channels
    gnw = nc.dram_tensor("gnw", [C], F32, kind="ExternalInput")
    gnb = nc.dram_tensor("gnb", [C], F32, kind="ExternalInput")
    qkb = nc.dram_tensor("qkb", [256], F32, kind="ExternalInput")        # 0:128 Q bias pair, 128:256 K bias pair
    if has_v_bias:
        vb = nc.dram_tensor("vb", [128], F32, kind="ExternalInput")
    yp = nc.dram_tensor("yp", [C, N], F32, kind="ExternalOutput")

    # group-indicator matrix: G[c, g] = 1 if c // 8 == g else 0  (per c-tile)
    g_host = np.zeros((128, 16), dtype=np.float32)
    for c in range(128):
        g_host[c, c // 8] = 1.0
    g_dram = nc.inline_tensor(g_host, name="gmat")

    with tile.TileContext(nc) as tc:
        with (
            tc.tile_pool(name="consts", bufs=1) as consts,
            tc.tile_pool(name="xh", bufs=2) as xh_pool,
            tc.tile_pool(name="hp", bufs=2) as hp_pool,
            tc.tile_pool(name="qk_sb", bufs=1) as qk_sb,
            tc.tile_pool(name="v1p", bufs=1) as v1p,
            tc.tile_pool(name="st", bufs=2) as st,
            tc.tile_pool(name="pexpp", bufs=3) as pexpp,
            tc.tile_pool(name="recp", bufs=4) as recp,
            tc.tile_pool(name="drec", bufs=4, space="DRAM") as drec,
            tc.tile_pool(name="rbp", bufs=4) as rbp,
            tc.tile_pool(name="yout", bufs=4) as yout,
        ):
            # ---- constants ----
            g_sb = consts.tile([128, 16], F32, tag="gsb")
            nc.sync.dma_start(out=g_sb, in_=g_dram[:, :])
            def load_weight(shape, dram_slice, tag):
                w_raw = consts.tile(shape, F32, tag=tag + "_r", name=tag + "_r")
                nc.sync.dma_start(out=w_raw, in_=dram_slice)
                if not f32r_mm:
                    return w_raw
                w_c = consts.tile(shape, MMD, tag=tag, name=tag)
                nc.vector.tensor_copy(w_c, w_raw)
                return w_c

            w_qk = [load_weight([128, 256], wqkT[t * 128:(t + 1) * 128, :], f"wqk{t}")
                    for t in range(2)]
            w_v = [load_weight([128, 128], wvT[t * 128:(t + 1) * 128, :], f"wv{t}")
                   for t in range(2)]
            wp_lo = load_weight([64, 256], wpT[0:64, :], "wplo")
            wp_hi = load_weight([64, 256], wpT[64:128, :], "wphi")

            gw_t, gb_t = [], []
            for t in range(2):
                gwt = consts.tile([128, 1], F32, tag=f"gw{t}")
                nc.sync.dma_start(out=gwt, in_=gnw[t * 128:(t + 1) * 128])
                gw_t.append(gwt)
                gbt = consts.tile([128, 1], F32, tag=f"gb{t}")
                nc.sync.dma_start(out=gbt, in_=gnb[t * 128:(t + 1) * 128])
                gb_t.append(gbt)
            bq = consts.tile([128, 1], F32, tag="bq")
            nc.sync.dma_start(out=bq, in_=qkb[0:128])
            bk = consts.tile([128, 1], F32, tag="bk")
            nc.sync.dma_start(out=bk, in_=qkb[128:256])
            if has_v_bias:
                vb_sb = consts.tile([128, 128], F32, tag="vbsb")
                nc.sync.dma_start(
                    out=vb_sb,
                    in_=bass.AP(tensor=vb, offset=0, ap=[[0, 128], [1, 128]]),
                )
            eps16 = consts.tile([16, 1], F32, tag="eps16")
            nc.vector.memset(eps16, EPS)

            # ---- phase A: GroupNorm (in place: x tile becomes h tile) ----
            h_t = []
            scale_t, bias_t = [], []
            with tc.tile_pool(name="ps_g", bufs=2, space="PSUM") as ps_g:
                for t in range(2):
                    x_t = xh_pool.tile([128, N], F32, tag="x", name=f"x{t}")
                    nc.sync.dma_start(out=x_t, in_=xb[t * 128:(t + 1) * 128, :])
                    h_t.append(x_t)

                    stats = st.tile([128, 8, 6], F32, tag="stats")
                    xr = x_t.rearrange("p (k f) -> p k f", f=512)
                    for k in range(8):
                        nc.vector.bn_stats(out=stats[:, k, :], in_=xr[:, k, :])
                    mv = st.tile([128, 2], F32, tag="mv")
                    nc.vector.bn_aggr(out=mv, in_=stats)

                    # mq = [mean_c, var_c + mean_c^2]
                    mq = st.tile([128, 2], F32, tag="mq")
                    nc.vector.tensor_copy(mq[:, 0:1], mv[:, 0:1])
                    sq = st.tile([128, 1], F32, tag="sq")
                    nc.vector.tensor_mul(sq, mv[:, 0:1], mv[:, 0:1])
                    nc.vector.tensor_add(mq[:, 1:2], mv[:, 1:2], sq)

                    gps = ps_g.tile([16, 2], F32, tag="gps")
                    nc.tensor.matmul(gps, lhsT=g_sb, rhs=mq, start=True, stop=True)
                    # per-group E[x], E[x^2]
                    gsb = st.tile([16, 2], F32, tag="gsb2")
                    nc.scalar.mul(gsb, gps, 0.125)
                    sqg = st.tile([16, 1], F32, tag="sqg")
                    nc.vector.tensor_mul(sqg, gsb[:, 0:1], gsb[:, 0:1])
                    var = st.tile([16, 1], F32, tag="var")
                    nc.vector.tensor_sub(var, gsb[:, 1:2], sqg)
                    std = st.tile([16, 1], F32, tag="std")
                    nc.scalar.activation(std, var, mybir.ActivationFunctionType.Sqrt,
                                         bias=eps16, scale=1.0)
                    rstd = st.tile([16, 1], F32, tag="rstd")
                    nc.vector.reciprocal(rstd, std)
                    ms = st.tile([16, 2], F32, tag=f"ms{t}")
                    nc.vector.tensor_copy(ms[:, 0:1], gsb[:, 0:1])
                    nc.vector.tensor_copy(ms[:, 1:2], rstd)

                    # broadcast per-group stats to per-channel [128,1] via DMA
                    bmean = st.tile([128, 1], F32, tag=f"bmean{t}")
                    src = ms[:, 0:1]
                    nc.sync.dma_start(
                        out=bmean,
                        in_=bass.AP(tensor=src.tensor, offset=src.offset,
                                    ap=[src.ap[0], [0, 8]]),
                    )
                    brstd = st.tile([128, 1], F32, tag=f"brstd{t}")
                    src2 = ms[:, 1:2]
                    nc.sync.dma_start(
                        out=brstd,
                        in_=bass.AP(tensor=src2.tensor, offset=src2.offset,
                                    ap=[src2.ap[0], [0, 8]]),
                    )
                    sc = consts.tile([128, 1], F32, tag=f"scale{t}")
                    nc.vector.tensor_mul(sc, brstd, gw_t[t])
                    scale_t.append(sc)
                    tmp = st.tile([128, 1], F32, tag="tmpb")
                    nc.vector.tensor_mul(tmp, bmean, sc)
                    bi = consts.tile([128, 1], F32, tag=f"bias{t}")
                    nc.vector.tensor_sub(bi, gb_t[t], tmp)
                    bias_t.append(bi)

                x_t_list = h_t
                h_t = []
                for t in range(2):
                    # h = x * scale + bias
                    h_new = hp_pool.tile([128, N], MMD, tag="h", name=f"h{t}")
                    nc.vector.tensor_scalar(
                        out=h_new, in0=x_t_list[t],
                        scalar1=scale_t[t], scalar2=bias_t[t],
                        op0=mybir.AluOpType.mult, op1=mybir.AluOpType.add,
                    )
                    h_t.append(h_new)

            # ---- phase B: QKV ----
            Q2 = qk_sb.tile([128, N], MAD, tag="q2")
            K2 = qk_sb.tile([128, N], MAD, tag="k2")
            V1 = v1p.tile([128, MB, 2, 65], MAD, tag="v1")
            with tc.tile_pool(name="ps_qkv", bufs=3, space="PSUM") as ps_qkv:
                for dst, col0, bias_ap in ((Q2, 0, bq), (K2, 128, bk)):
                    for nb in range(NB):
                        ps = ps_qkv.tile([128, 512], F32, tag="mm")
                        ns = slice(nb * 512, (nb + 1) * 512)
                        nc.tensor.matmul(ps, lhsT=_cast(w_qk[0][:, col0:col0 + 128], f32r_mm),
                                         rhs=_cast(h_t[0][:, ns], f32r_mm), start=True, stop=False)
                        nc.tensor.matmul(ps, lhsT=_cast(w_qk[1][:, col0:col0 + 128], f32r_mm),
                                         rhs=_cast(h_t[1][:, ns], f32r_mm), start=False, stop=True)
                        nc.vector.tensor_scalar(
                            out=dst[:, ns], in0=ps, scalar1=bias_ap, scalar2=None,
                            op0=mybir.AluOpType.add,
                        )
                ones1 = consts.tile([128, 1], F32, tag="ones1")
                nc.vector.memset(ones1, 1.0)
                nc.vector.tensor_copy(
                    V1[:, :, :, 64:65].rearrange("p a b o -> p (a b o)"),
                    ones1.to_broadcast((128, MB * 2)),
                )
                for ch in range(MB):
                    ps = ps_qkv.tile([128, 128], F32, tag="mm")
                    cs = slice(ch * 128, (ch + 1) * 128)
                    nc.tensor.matmul(ps, lhsT=_cast(h_t[0][:, cs], f32r_mm),
                                     rhs=w_v[0], start=True, stop=False)
                    nc.tensor.matmul(ps, lhsT=_cast(h_t[1][:, cs], f32r_mm),
                                     rhs=w_v[1], start=False, stop=True)
                    if has_v_bias:
                        nc.vector.tensor_add(
                            V1[:, ch, :, 0:64],
                            ps.rearrange("p (h d) -> p h d", h=2),
                            vb_sb.rearrange("p (h d) -> p h d", h=2),
                        )
                    else:
                        nc.vector.tensor_copy(
                            V1[:, ch, :, 0:64],
                            ps.rearrange("p (h d) -> p h d", h=2),
                        )

            # ---- phase C: attention ----
            A_lo = xh_pool.tile([64, N], MMD, tag="x", name="A_lo")
            A_hi = xh_pool.tile([64, N], MMD, tag="x", name="A_hi")
            with (
                tc.tile_pool(name="ps_qk", bufs=2, space="PSUM") as ps_qk,
                tc.tile_pool(name="ps_pv", bufs=4, space="PSUM") as ps_pv,
            ):
                def emit_qk(nb, mb):
                    ns = slice(nb * 512, (nb + 1) * 512)
                    ms_ = slice(mb * 128, (mb + 1) * 128)
                    qk = ps_qk.tile([128, 1024], F32, tag="qk", name=f"qk_{nb}_{mb}")
                    nc.tensor.matmul(qk[:, 0:512], lhsT=_cast(K2[0:64, ms_], f32r_attn),
                                     rhs=_cast(Q2[0:64, ns], f32r_attn), start=True, stop=True,
                                     skip_group_check=True)
                    nc.tensor.matmul(qk[:, 512:1024], lhsT=_cast(K2[64:128, ms_], f32r_attn),
                                     rhs=_cast(Q2[64:128, ns], f32r_attn), start=True, stop=True,
                                     skip_group_check=True)
                    return qk

                # software pipeline: emit iteration i+1's QK matmuls before
                # iteration i's PV matmuls, so the in-order PE queue never
                # stalls behind a PV that waits on ScalarE's exp.
                iters = [(nb, mb) for nb in range(NB) for mb in range(MB)]
                pv_tiles = {}
                qk_cur = emit_qk(*iters[0])
                for idx, (nb, mb) in enumerate(iters):
                    ns = slice(nb * 512, (nb + 1) * 512)
                    if mb == 0:
                        pv_lo = ps_pv.tile([65, 512], F32, tag="pv", name=f"pvlo_{nb}")
                        pv_hi = ps_pv.tile([65, 512], F32, tag="pv", name=f"pvhi_{nb}")
                        pv_tiles[nb] = (pv_lo, pv_hi)
                    pv_lo, pv_hi = pv_tiles[nb]
                    pexp = pexpp.tile([128, 1024], MAD, tag="pexp", name=f"pexp_{nb}_{mb}")
                    nc.scalar.activation(pexp, qk_cur,
                                         mybir.ActivationFunctionType.Exp,
                                         scale=SCALE)
                    if idx + 1 < len(iters):
                        qk_cur = emit_qk(*iters[idx + 1])
                    nc.tensor.matmul(pv_lo, lhsT=_cast(V1[:, mb, 0, :], f32r_attn),
                                     rhs=_cast(pexp[:, 0:512], f32r_attn),
                                     start=(mb == 0), stop=(mb == MB - 1),
                                     skip_group_check=True)
                    nc.tensor.matmul(pv_hi, lhsT=_cast(V1[:, mb, 1, :], f32r_attn),
                                     rhs=_cast(pexp[:, 512:1024], f32r_attn),
                                     start=(mb == 0), stop=(mb == MB - 1),
                                     skip_group_check=True)
                    if mb == MB - 1:
                        # normalize: A = pv[0:64] / pv[64]
                        for pv, A in ((pv_lo, A_lo), (pv_hi, A_hi)):
                            rec = recp.tile([65, 512], F32, tag="rec")
                            nc.vector.reciprocal(rec[64:65, :], pv[64:65, :])
                            # bounce through DRAM: SBUF APs cannot have a
                            # zero-step partition dim, DRAM APs can.
                            dr = drec.tile([1, 512], F32, tag="dr")
                            nc.sync.dma_start(out=dr, in_=rec[64:65, :])
                            rb = rbp.tile([64, 512], F32, tag="rb")
                            nc.sync.dma_start(
                                out=rb,
                                in_=bass.AP(tensor=dr.tensor, offset=dr.offset,
                                            ap=[[0, 64]] + list(dr.ap[1:])),
                            )
                            nc.vector.tensor_mul(A[:, ns], pv[0:64, :], rb)

            # ---- phase D: partial proj ----
            with tc.tile_pool(name="ps_pj", bufs=2, space="PSUM") as ps_pj:
                for m in range(2):
                    for nb in range(NB):
                        ns = slice(nb * 512, (nb + 1) * 512)
                        ps = ps_pj.tile([128, 512], F32, tag="pj")
                        nc.tensor.matmul(ps, lhsT=_cast(wp_lo[:, m * 128:(m + 1) * 128], f32r_mm),
                                         rhs=_cast(A_lo[:, ns], f32r_mm), start=True, stop=False)
                        nc.tensor.matmul(ps, lhsT=_cast(wp_hi[:, m * 128:(m + 1) * 128], f32r_mm),
                                         rhs=_cast(A_hi[:, ns], f32r_mm), start=False, stop=True)
                        y_sb = yout.tile([128, 512], F32, tag="y")
                        nc.scalar.copy(y_sb, ps)
                        nc.sync.dma_start(out=yp[m * 128:(m + 1) * 128, ns], in_=y_sb)

    nc.finalize()
    return nc


_CACHE = {}


F32R_ATTN = True
F32R_MM = True


def _get_program(has_v_bias: bool):
    key = ("prog", has_v_bias, F32R_ATTN, F32R_MM)
    if key not in _CACHE:
        _CACHE[key] = _build_program(has_v_bias, F32R_ATTN, F32R_MM)
    return _CACHE[key]


def _make_in_maps(x, gn_w, gn_b, qkv_w, qkv_b, proj_w):
    x = np.ascontiguousarray(x, dtype=np.float32)
    in_maps = []
    for core in range(NCORES):
        b, p = core // 2, core % 2
        rows_q = slice(p * 128, (p + 1) * 128)
        rows_k = slice(256 + p * 128, 256 + (p + 1) * 128)
        rows_v = slice(512 + p * 128, 512 + (p + 1) * 128)
        m = {
            "xb": np.ascontiguousarray(x[b].reshape(C, N)),
            "wqkT": np.ascontiguousarray(
                np.concatenate([qkv_w[rows_q], qkv_w[rows_k]], axis=0).T.astype(np.float32)),
            "wvT": np.ascontiguousarray(qkv_w[rows_v].T.astype(np.float32)),
            "wpT": np.ascontiguousarray(proj_w[:, p * 128:(p + 1) * 128].T.astype(np.float32)),
            "gnw": np.ascontiguousarray(gn_w.astype(np.float32)),
            "gnb": np.ascontiguousarray(gn_b.astype(np.float32)),
            "qkb": np.ascontiguousarray(
                np.concatenate([qkv_b[rows_q], qkv_b[rows_k]]).astype(np.float32)),
        }
        if np.any(qkv_b[512:768]):
            m["vb"] = np.ascontiguousarray(qkv_b[rows_v].astype(np.float32))
        in_maps.append(m)
    return in_maps


def _get_executor(nc, chain=1):
    """Build (once) a cached jitted 8-core executor for the program.

    Mirrors concourse.bass2jax.run_bass_via_pjrt, but caches the jitted
    callable so repeat kernel() calls don't re-trace/re-compile the XLA
    wrapper.  Returns (fn, in_names, out_names) where fn takes a list of
    per-core input dicts and returns a list of per-core output dicts.
    """
    key = ("exec", id(nc), chain)
    if key in _CACHE:
        return _CACHE[key]
    import jax
    import concourse.mybir as _mybir
    from jax.experimental.shard_map import shard_map
    from jax.sharding import Mesh, PartitionSpec
    from concourse import bass2jax

    bass2jax.install_neuronx_cc_hook()
    partition_name = nc.partition_id_tensor.name if nc.partition_id_tensor else None
    in_names, out_names, out_avals, zero_outs = [], [], [], []
    for alloc in nc.m.functions[0].allocations:
        if not isinstance(alloc, _mybir.MemoryLocationSet):
            continue
        name = alloc.memorylocations[0].name
        if alloc.kind == "ExternalInput":
            if name != partition_name:
                in_names.append(name)
        elif alloc.kind == "ExternalOutput":
            shape = tuple(alloc.tensor_shape)
            dtype = _mybir.dt.np(alloc.dtype)
            out_names.append(name)
            out_avals.append(jax.core.ShapedArray(shape, dtype))
            zero_outs.append(np.zeros(shape, dtype))
    n_params = len(in_names)
    n_outs = len(out_avals)
    all_names = in_names + out_names + ([partition_name] if partition_name else [])

    def _body(*args):
        ins = list(args[:n_params])
        outs = list(args[n_params:])
        pid = [bass2jax.partition_id_tensor()] if partition_name is not None else []
        # chain > 1 re-runs the NEFF, feeding the previous run's outputs as
        # the next run's (donated) output buffers: a data dependency that
        # serializes runs and defeats CSE, for marginal-time benchmarking.
        for _ in range(chain):
            outs = list(bass2jax._bass_exec_p.bind(
                *ins, *outs, *pid,
                out_avals=tuple(out_avals),
                in_names=tuple(all_names),
                out_names=tuple(out_names),
                lowering_input_output_aliases=(),
                sim_require_finite=True,
                sim_require_nnan=True,
                nc=nc,
            ))
        return tuple(outs)

    devices = jax.devices()[:NCORES]
    mesh = Mesh(np.asarray(devices), ("core",))
    in_specs = (PartitionSpec("core"),) * (n_params + n_outs)
    out_specs = (PartitionSpec("core"),) * n_outs
    donate = tuple(range(n_params, n_params + n_outs))
    sharded = jax.jit(
        shard_map(_body, mesh=mesh, in_specs=in_specs, out_specs=out_specs,
                  check_rep=False),
        donate_argnums=donate, keep_unused=True,
    )

    def fn(in_maps):
        concat_in = [
            np.concatenate([np.asarray(in_maps[c][nm]) for c in range(NCORES)], axis=0)
            for nm in in_names
        ]
        concat_zeros = [
            np.zeros((NCORES * z.shape[0], *z.shape[1:]), z.dtype) for z in zero_outs
        ]
        out_arrs = sharded(*concat_in, *concat_zeros)
        return [
            {nm: np.asarray(out_arrs[i]).reshape(NCORES, *out_avals[i].shape)[c]
             for i, nm in enumerate(out_names)}
            for c in range(NCORES)
        ]

    _CACHE[key] = (fn, in_names, out_names)
    return _CACHE[key]


def _prep(inputs):
    x = np.asarray(inputs["x"], dtype=np.float32)
    qkv_b = np.asarray(inputs["qkv_b"], dtype=np.float32)
    has_v_bias = bool(np.any(qkv_b[512:768]))
    nc = _get_program(has_v_bias)
    in_maps = _make_in_maps(
        x,
        np.asarray(inputs["gn_w"], dtype=np.float32),
        np.asarray(inputs["gn_b"], dtype=np.float32),
        np.asarray(inputs["qkv_w"], dtype=np.float32),
        qkv_b,
        np.asarray(inputs["proj_w"], dtype=np.float32),
    )
    return nc, in_maps, x


def run(inputs, trace=False):
    """Run the sharded kernel.  Returns (output, per-core results list)."""
    nc, in_maps, x = _prep(inputs)
    fn, _, _ = _get_executor(nc)
    results = fn(in_maps)
    proj_b = np.asarray(inputs["proj_b"], dtype=np.float32)
    parts = [results[c]["yp"] for c in range(NCORES)]
    y = np.stack([parts[2 * b] + parts[2 * b + 1] for b in range(B)])  # [B, C, N]
    y = y + proj_b[None, :, None]
    out = np.asarray(inputs["x"], dtype=np.float32) + y.reshape(B, C, 64, 64)
    return out.astype(np.float32), results


def bench(inputs, n=9, reps=3):
    """Marginal per-execution device time via chained NEFF runs.

    Times a jit that executes the NEFF once vs `n` times back-to-back
    (serialized through donated output buffers); the marginal time
    (T_n - T_1) / (n - 1) excludes host<->device transfer and dispatch.
    """
    import time as _time
    nc, in_maps, _ = _prep(inputs)
    fn1, _, _ = _get_executor(nc, chain=1)
    fnN, _, _ = _get_executor(nc, chain=n)

    def timed(f):
        f(in_maps)  # warm (compile)
        best = float("inf")
        for _ in range(reps):
            t0 = _time.perf_counter()
            f(in_maps)
            best = min(best, _time.perf_counter() - t0)
        return best

    t1, tn = timed(fn1), timed(fnN)
    return (tn - t1) / (n - 1), t1, tn


def kernel(**inputs) -> np.ndarray:
    out, _ = run(inputs, trace=False)
    return out
